# revision 57
# baseline (speedup 1.0000x reference)
"""Trainium2 Bass kernel for the sparse-attention scoring module.

Reference computation (S=2048, B=32, H=1024):
    energy[s,b,:]   = enc[s,b,:] @ W^T + bias            # [S,B,H]
    attn[b,s]       = hidden[b,:] . energy[s,b,:]        # [B,S]
    out             = softmax(attn, axis=1)[None]        # [1,B,S]

Algebraic rewrite used here:
    attn[b,s] = (hidden[b,:] @ W) . enc[s,b,:]  +  hidden[b,:] . bias
The bias term is constant per row b and cancels in the softmax, so the
kernel never touches `b`.  The [S,B,H] x [H,H] matmul (137 GFLOP)
collapses to a [B,H] x [H,H] matmul plus a batched dot-product, making
the kernel memory-bound on streaming encoder_outputs (268 MB) once.

Sharding: data-parallel over batch.  Each of the 8 cores gets 4 batches:
    enc shard [2048, 4, 1024], hidden shard [4, 1024], full W.
Per core:
    - prologue: v = hidden @ W on TensorE (hidden transposed on-chip),
      then vbcat[p, b*H+h] = v[b, h] replicated to all 128 partitions via
      K=4 selector matmuls (identity-column lhsT broadcast along free).
    - main loop: 16 s-tiles [128, 4*1024] streamed by DMA at ~460 GB/s;
      per tile ONE wide VectorE multiply (et * vbcat), then per b a
      ScalarE activation(Copy, accum_out=...) row-reduction into
      logits[b][:, t] (a few go to VectorE tensor_reduce for balance).
      DMA / DVE / ACT each sit at ~70-90 us per pass and overlap.
    - tail: softmax over the 2048 logits per b: per-partition max on DVE,
      global max via TensorE transpose + reduce, broadcast back with a
      diag+ones matmul, exp+rowsum on ScalarE, denominator via
      sumexp^T @ ones matmul, reciprocal on DVE, scale, TensorE
      transpose, single contiguous store.  No GPSIMD in the hot path.
"""

import sys

if "/opt/trn_rl_repo" not in sys.path:
    sys.path.insert(0, "/opt/trn_rl_repo")

import numpy as np

import concourse.bass as bass
import concourse.mybir as mybir
import concourse.tile as tile
from concourse import bacc, bass_utils
from concourse.bass import ts
from concourse.bass_isa import ReduceOp
from concourse.masks import make_identity

S, B, H = 2048, 32, 1024
NCORES = 8
BS = B // NCORES  # 4 batches per core
P = 128
T = S // P  # 16 s-tiles per core
KC = H // P  # 8 contraction chunks
F32 = mybir.dt.float32
AX = mybir.AxisListType
ALU = mybir.AluOpType
ACT = mybir.ActivationFunctionType

ENC_BUFS = 6
PROD_BUFS = 3


def build_kernel_body(
    nc, tc, enc, hid_d, w_d, out_d, repeat=1, variant="full", et_alt=False,
    gps=False, batch2=False, w_ring="scalar", proto_in_loop=False,
):
    """Emit the per-core program.  enc/hid_d/w_d/out_d are DRAM APs.

    repeat > 1 re-runs the main streaming loop (for timing calibration
    only -- logits are simply overwritten, output stays correct).
    variant: "full" | "dmaonly" (skip compute) | "computeonly" (skip DMA),
    both for bottleneck isolation; their outputs are garbage."""
    import contextlib

    with contextlib.ExitStack() as ctx:
        consts = ctx.enter_context(tc.tile_pool(name="consts", bufs=1))
        wpool = ctx.enter_context(tc.tile_pool(name="wpool", bufs=1))
        epool = ctx.enter_context(
            tc.tile_pool(name="epool", bufs=3 if batch2 else ENC_BUFS)
        )
        vbpool = ctx.enter_context(tc.tile_pool(name="vbpool", bufs=1))
        prodpool = ctx.enter_context(tc.tile_pool(name="prodpool", bufs=PROD_BUFS))
        small = ctx.enter_context(tc.tile_pool(name="small", bufs=1))
        ptp = ctx.enter_context(tc.tile_pool(name="ptp", bufs=2, space="PSUM"))
        pvb = ctx.enter_context(tc.tile_pool(name="pvb", bufs=2, space="PSUM"))
        ptail = ctx.enter_context(tc.tile_pool(name="ptail", bufs=1, space="PSUM"))

        identity = consts.tile([P, P], F32)
        make_identity(nc, identity)
        ones = consts.tile([P, 1], F32)
        nc.gpsimd.memset(ones, 1.0)
        ones4 = consts.tile([BS, P], F32)
        nc.gpsimd.memset(ones4, 1.0)

        # Preload the exp activation table so the tail doesn't pay ~2.7us.
        warm = small.tile([1, 1], F32)
        nc.gpsimd.memset(warm, 1.0)
        nc.scalar.activation(warm, warm, ACT.Exp)

        # ---- prologue: hidden, W, and the broadcast v tiles ----
        # hid/W go on the ACT-issued HWDGE ring: ACT is idle during the
        # prologue (the +9us/pass interference seen when streaming et from
        # this ring does not apply here), and it frees the sync ring so the
        # enc stream starts at t=0 instead of behind 4 MB of W -- worth
        # ~14 us of single-call device time.
        w_view = w_d.rearrange("(kc kp) h -> kc kp h", kp=P)  # [8, 128, 1024]
        wdma = nc.scalar if w_ring == "scalar" else nc.sync
        vbcat = vbpool.tile([P, BS * H], F32)

        def emit_prologue():
            hid = small.tile([BS, H], F32, tag="hid", name="hid")
            wdma.dma_start(hid, hid_d)

            # hidT[:, 4*kc + b] = hidden[b, kc*128 : (kc+1)*128]
            hidT = small.tile([P, BS * KC], F32, tag="hidT", name="hidT")
            for kc in range(KC):
                pt = ptp.tile([P, BS], F32, tag="pt", name="pt")
                nc.tensor.transpose(pt, hid[:, ts(kc, P)], identity[0:BS, 0:BS])
                nc.scalar.copy(hidT[:, ts(kc, BS)], pt)

            wt = []
            for kc in range(KC):
                wtile = wpool.tile([P, H], F32, tag=f"w{kc}", name=f"w{kc}")
                wdma.dma_start(wtile, w_view[kc])
                wt.append(wtile)

            # v[b, :] = hidden[b] @ W, computed once (out partitions 0..3) ...
            v = small.tile([BS, H], F32, tag="v", name="v")
            for hh in range(2):
                accv = pvb.tile([BS, 512], F32, tag="acc", name="accv")
                for kc in range(KC):
                    nc.tensor.matmul(
                        accv,
                        lhsT=hidT[:, ts(kc, BS)],
                        rhs=wt[kc][:, ts(hh, 512)],
                        start=(kc == 0),
                        stop=(kc == KC - 1),
                    )
                nc.scalar.copy(v[:, ts(hh, 512)], accv)

            # ... then vbcat[p, b*H + h] = v[b, h] for every p: row b of v
            # replicated on all 128 partitions via a K=4 selector matmul,
            # lhsT[k, p] = identity[k, b] (= 1 iff k == b) broadcast along
            # free.  vbcat matches the et layout: ONE wide DVE mult per tile.
            for b in range(BS):
                for hh in range(2):
                    acc = pvb.tile([P, 512], F32, tag="acc", name="acc")
                    nc.tensor.matmul(
                        acc,
                        lhsT=identity[0:BS, b : b + 1].broadcast_to([BS, P]),
                        rhs=v[:, ts(hh, 512)],
                        start=True,
                        stop=True,
                    )
                    nc.scalar.copy(vbcat[:, ts(2 * b + hh, 512)], acc)

        if not proto_in_loop:
            emit_prologue()

        # ---- main loop: fused dot-products over the enc stream ----
        # s-tile t covers s in [128t, 128t+128), partition p <-> s = 128t + p.
        enc_view = enc.rearrange("(t p) b h -> t p (b h)", p=P)  # [16, 128, 4096]
        # batch2: one 4 MB DMA covers s-tiles 2u and 2u+1 (halves the DMA
        # issue/completion count; descriptor sizes are unchanged at 16 KB).
        enc_view2 = enc.rearrange("(u c p) b h -> u p c (b h)", c=2, p=P)
        logits = [
            small.tile([P, T], F32, tag=f"lg{b}", name=f"lg{b}") for b in range(BS)
        ]
        # DVE does the elementwise multiply; ScalarE reduces each product row
        # via activation(Copy, accum_out=...) -> 2-stage cross-engine pipeline.
        # ACT's throwaway output goes to PSUM (ScalarE is closer to PSUM).
        dumpp = ptail.tile([P, 1024], F32, tag="dumpp", name="dumpp")
        dump = small.tile([P, H], F32)
        if variant in ("dmaonly", "noact"):
            for b in range(BS):
                nc.vector.memset(logits[b], 0.0)
        cet = None
        if variant == "computeonly":
            cet = epool.tile([P, BS * H], F32, tag="et", name="cet")
            nc.gpsimd.memset(cet, 0.001)
        # width: full kernel reduces H elements per (t, b); halfwidth reduces
        # H/2 with identical instruction counts (overhead calibration only).
        wid = H // 2 if variant == "halfwidth" else H
        for _rep in range(repeat):
            if proto_in_loop:
                emit_prologue()
            et2 = None
            for t in range(T):
                if variant == "computeonly":
                    et = cet
                elif batch2:
                    if t % 2 == 0:
                        et2 = epool.tile(
                            [P, 2 * BS * H], F32, tag="et", name="et2"
                        )
                        nc.sync.dma_start(et2, enc_view2[t // 2])
                    et = et2[:, ts(t % 2, BS * H)]
                else:
                    et = epool.tile([P, BS * wid], F32, tag="et", name="et")
                    eng = nc.scalar if (et_alt and t % 2 == 1) else nc.sync
                    if variant == "halfwidth":
                        eng.dma_start(et, enc_view[t][:, 0 : BS * wid])
                    else:
                        eng.dma_start(et, enc_view[t])
                if variant == "dmaonly":
                    # touch one column so DCE keeps the DMA
                    nc.vector.tensor_scalar_mul(dump[:, 0:1], et[:, 0:1], 1.0)
                    continue
                if variant == "nodve":
                    for b in range(BS):
                        # ACT accumulates straight from et (no multiply)
                        nc.scalar.activation(
                            dump,
                            et[:, ts(b, H)],
                            ACT.Copy,
                            accum_out=logits[b][:, t : t + 1],
                        )
                    continue
                prod = prodpool.tile([P, BS * wid], F32, tag="prod", name="prod")
                # optionally offload some multiplies to the (otherwise idle)
                # GPSIMD engine to unload DVE
                if gps and t % 5 == 2:
                    nc.gpsimd.tensor_mul(prod, et, vbcat[:, 0 : BS * wid])
                else:
                    nc.vector.tensor_mul(prod, et, vbcat[:, 0 : BS * wid])
                if variant == "noact":
                    continue
                for b in range(BS):
                    # ACT does most reductions; hand a few to DVE to balance
                    # the two engines (both sit just above the DMA floor).
                    dve_red = (b == 3 and t % 2 == 1) if gps else (
                        b == 3 and t % 4 == 1
                    )
                    if dve_red:
                        nc.vector.tensor_reduce(
                            logits[b][:, t : t + 1],
                            prod[:, ts(b, wid)],
                            axis=AX.X,
                            op=ALU.add,
                        )
                    else:
                        nc.scalar.activation(
                            dumpp[:, 0:wid],
                            prod[:, ts(b, wid)],
                            ACT.Copy,
                            accum_out=logits[b][:, t : t + 1],
                        )

        # ---- tail: softmax over s (2048 values per b), two-pass exp ----
        # pass 1 computes exp(x - M1) and its sums (M1 = per-partition max,
        # shifted enough for fp range); pass 2 re-exps with the exact bias
        # -M1b - ln(sum exp(x - M1b)), which folds the normalization in.
        m1 = small.tile([P, BS], F32)
        for b in range(BS):
            nc.vector.tensor_reduce(
                m1[:, b : b + 1], logits[b], axis=AX.X, op=ALU.max
            )
        # global per-b max via TensorE transpose + free-dim reduce
        pm1t = ptp.tile([BS, P], F32, tag="pt", name="pm1t")
        nc.tensor.transpose(pm1t, m1, identity)
        mx4 = small.tile([BS, 1], F32)
        nc.vector.tensor_reduce(mx4, pm1t, axis=AX.X, op=ALU.max)
        # broadcast -mx4 to all partitions: diag(-mx4) then ones^T @ diag
        dg4 = small.tile([BS, BS], F32)
        nc.vector.tensor_scalar(
            dg4,
            identity[0:BS, 0:BS],
            mx4,
            -1.0,
            op0=ALU.mult,
            op1=ALU.mult,
        )
        pneg = ptp.tile([P, BS], F32, tag="pt", name="pneg")
        nc.tensor.matmul(pneg, lhsT=ones4, rhs=dg4, start=True, stop=True)
        negmax = small.tile([P, BS], F32)
        nc.scalar.copy(negmax, pneg)

        probs = small.tile([P, BS * T], F32)  # [128, 64], col = b*16 + t
        sumexp = small.tile([P, BS], F32)
        for b in range(BS):
            nc.scalar.activation(
                probs[:, ts(b, T)],
                logits[b],
                ACT.Exp,
                bias=negmax[:, b : b + 1],
                scale=1.0,
                accum_out=sumexp[:, b : b + 1],
            )

        # denominator, directly transposed: den[b] = sum_p sumexp[p, b]
        # via lhsT=sumexp, rhs=ones -> [4, 1]; reciprocal on DVE, then
        # broadcast 1/den to all partitions with the diag + ones matmul
        # and scale probs per b.
        pdent = ptp.tile([BS, 1], F32, tag="pt", name="pdent")
        nc.tensor.matmul(pdent, lhsT=sumexp, rhs=ones, start=True, stop=True)
        rdent = small.tile([BS, 1], F32)
        nc.vector.reciprocal(rdent, pdent)
        dg4b = small.tile([BS, BS], F32)
        nc.vector.tensor_scalar(
            dg4b,
            identity[0:BS, 0:BS],
            rdent,
            None,
            op0=ALU.mult,
        )
        pb2 = ptp.tile([P, BS], F32, tag="pt", name="pb2")
        nc.tensor.matmul(pb2, lhsT=ones4, rhs=dg4b, start=True, stop=True)
        rbc = small.tile([P, BS], F32)
        nc.scalar.copy(rbc, pb2)
        for b in range(BS):
            nc.vector.tensor_scalar_mul(
                probs[:, ts(b, T)], probs[:, ts(b, T)], rbc[:, b : b + 1]
            )

        # ---- store: transpose so DRAM rows are contiguous ----
        # probs[p, b*16+t] -> oT[b*16+t, p];  out[b, 128t + p] = oT[(b,t), p]
        poT = ptail.tile([BS * T, P], F32, tag="poT")
        nc.tensor.transpose(poT, probs, identity)
        oT = small.tile([BS * T, P], F32)
        nc.scalar.copy(oT, poT)
        out_view = out_d.rearrange("b (t p) -> (b t) p", p=P)  # [64, 128]
        nc.sync.dma_start(out_view, oT)


def build_nc(repeat=1, variant="full", et_alt=False, gps=False, batch2=False,
             w_ring="scalar", proto_in_loop=False):
    nc = bacc.Bacc(
        "TRN2",
        target_bir_lowering=False,
        debug=False,
        num_devices=NCORES,
    )
    enc = nc.dram_tensor("enc", [S, BS, H], F32, kind="ExternalInput").ap()
    hid_d = nc.dram_tensor("hidden", [BS, H], F32, kind="ExternalInput").ap()
    w_d = nc.dram_tensor("w", [H, H], F32, kind="ExternalInput").ap()
    out_d = nc.dram_tensor("out", [BS, S], F32, kind="ExternalOutput").ap()
    with tile.TileContext(nc) as tc:
        build_kernel_body(
            nc, tc, enc, hid_d, w_d, out_d, repeat=repeat, variant=variant,
            et_alt=et_alt, gps=gps, batch2=batch2, w_ring=w_ring,
            proto_in_loop=proto_in_loop,
        )
    nc.compile()
    return nc


def make_in_maps(hidden, encoder_outputs, W):
    hidden = np.asarray(hidden, dtype=np.float32)
    encoder_outputs = np.asarray(encoder_outputs, dtype=np.float32)
    W = np.ascontiguousarray(np.asarray(W, dtype=np.float32))
    in_maps = []
    for c in range(NCORES):
        in_maps.append(
            {
                "enc": np.ascontiguousarray(
                    encoder_outputs[:, c * BS : (c + 1) * BS, :]
                ),
                "hidden": np.ascontiguousarray(hidden[c * BS : (c + 1) * BS, :]),
                "w": W,
            }
        )
    return in_maps


_NC_CACHE = {}


def get_nc():
    if "nc" not in _NC_CACHE:
        _NC_CACHE["nc"] = build_nc()
    return _NC_CACHE["nc"]


def kernel(hidden, encoder_outputs, W, b, **_unused):
    # The linear-layer bias contributes hidden[b].bias to every logit of
    # row b, a per-row constant that cancels in the softmax -> unused.
    nc = get_nc()
    in_maps = make_in_maps(hidden, encoder_outputs, W)
    res = bass_utils.run_bass_kernel_spmd(
        nc, in_maps, core_ids=list(range(NCORES))
    )
    outs = [res.results[c]["out"] for c in range(NCORES)]
    full = np.concatenate(outs, axis=0)  # [32, 2048]
    return full[None, :, :].astype(np.float32, copy=False)
